# revision 10
# baseline (speedup 1.0000x reference)
"""GQA kernel v4 for Trainium2 (Bass/Tile), 8 NeuronCores.

Sharding: core c -> batch b=c//4, kv-head pair j=c%4 (kv heads 2j,2j+1,
q heads 8j..8j+7).  Each core computes out[b, :, 512j:512(j+1)] (pair-major
column order + per-head softmax denominator; divided + untangled on host).

v4 structure (trace-driven, from the 288us baseline):
  - ONE attention block per head-pair (q-block = full T): bands ki=0..15,
    each band is S^T(ki) over q cols [128ki, T).  PV accumulation group
    for q-tile qt is woven right after band qt+LAG, so PV matmuls spread
    across the whole pair instead of bursting at block end (the v3
    qb-split left 10-band "deserts" with no PE work -> HAM half-clock).
  - PV regions live in a ring of [128,512] PSUM bank tiles (7 x 65-wide
    regions each, bufs=2); each group's [128,65] numerator+denominator
    is copied to SBUF by GPSIMD right after the group closes (ACT/DVE
    stay free for exp; no end-of-block copy chain).
  - S PSUM ring (tag "s", bufs=4, 512-wide chunks) decouples S matmuls
    from exp consumption.
  - Input DMA split across the two HWDGE queues (sync + scalar),
    priority-ordered so phase A streams just-in-time.
  - PE warm-up matmuls at t=0 (triu x triu) warm the HAM clock gate.
  - RoPE partition swap via a signed-permutation matmul on the PE.
  - Softmax division on the host (numerators + ones-column denominator).
  - exp split per band: head m via ACT (exact), head m+4 via DVE int16
    Schraudolph writing bf16 bit patterns.
"""

import sys

for _p in ("/opt/trn_rl_repo",):
    if _p not in sys.path:
        sys.path.insert(0, _p)

import contextlib

import numpy as np
import ml_dtypes

import concourse.bass as bass
import concourse.tile as tile
from concourse import bacc, mybir
from concourse.bass_utils import run_bass_kernel_spmd
from concourse.masks import make_upper_triangular

BF16 = mybir.dt.bfloat16
F32 = mybir.dt.float32
I16 = mybir.dt.int16
AF = mybir.ActivationFunctionType
ALU = mybir.AluOpType

D = 2048
HS = 64
SCALE = 1.0 / 8.0  # 1/sqrt(HS)

EXP_A = float(128.0 * 1.4426950408889634 * SCALE)
EXP_B = float(127.0 * 128.0 - 366393.0 / 65536.0 - 0.5)

WARMUP_MMS = 44
LAG = 2


def _emit_body(tc, aps, T):
    nc = tc.nc
    NT = T // 128            # k/q tiles
    ND = D // 128            # contraction chunks
    TCW = min(512, T)        # projection t-chunk width
    NTC = T // TCW
    SCW = min(512, T)        # S-chunk width
    HT = T // 2

    # pair 0 is split into two q-blocks so attention starts as soon as
    # the first half of x/Q0 is resident; pairs 1-3 are single blocks
    BLOCKS = [(0, 0, HT), (0, HT, T), (1, 0, T), (2, 0, T), (3, 0, T)]
    block_base = {}
    _g = 0
    for bi, (m, qlo, qhi) in enumerate(BLOCKS):
        block_base[bi] = _g
        _g += qhi // 128
    NBANDS = _g

    xT, wqT, wkT, wvT, cosr, sins, permM, out = aps

    ctx = tc._kernel_exitstack = contextlib.ExitStack()

    pers = ctx.enter_context(tc.tile_pool(name="pers", bufs=1))
    rp = ctx.enter_context(tc.tile_pool(name="rope", bufs=2))
    ppts = ctx.enter_context(tc.tile_pool(name="ppts", bufs=1))
    stgp = ctx.enter_context(tc.tile_pool(name="stgp", bufs=1))
    qpool = ctx.enter_context(tc.tile_pool(name="qpool", bufs=2))

    # ---- persistent tiles ----
    xsb = pers.tile([128, ND, T], BF16, tag="xsb")
    wqsb = pers.tile([128, ND, 512], BF16, tag="wqsb")
    wksb = pers.tile([128, ND, 128], BF16, tag="wksb")
    wvsb = pers.tile([128, ND, 128], BF16, tag="wvsb")
    xTs = [xsb[:, di, :] for di in range(ND)]
    wqTs = [wqsb[:, di, :] for di in range(ND)]
    wkTs = [wksb[:, di, :] for di in range(ND)]
    wvTs = [wvsb[:, di, :] for di in range(ND)]
    cosr_t = pers.tile([128, T], BF16, tag="cosr")
    sins_t = pers.tile([128, T], BF16, tag="sins")
    perm_t = pers.tile([128, 128], BF16, tag="perm")

    # ---- input DMA: two HWDGE queues, ordered by first-use time ----
    hnd = ND // 2
    qnd = ND // 4
    nc.scalar.dma_start(out=wksb[:], in_=wkT[:, :, :])
    nc.sync.dma_start(out=xsb[:, 0:qnd, 0:TCW], in_=xT[:, 0:qnd, 0:TCW])
    nc.sync.dma_start(out=xsb[:, qnd:hnd, 0:TCW], in_=xT[:, qnd:hnd, 0:TCW])
    nc.scalar.dma_start(out=xsb[:, hnd:3 * qnd, 0:TCW],
                        in_=xT[:, hnd:3 * qnd, 0:TCW])
    nc.scalar.dma_start(out=xsb[:, 3 * qnd:ND, 0:TCW],
                        in_=xT[:, 3 * qnd:ND, 0:TCW])
    nc.sync.dma_start(out=wqsb[:, :, 0:128], in_=wqT[:, :, 0:128])
    if T > TCW:
        nc.sync.dma_start(out=xsb[:, 0:hnd, TCW:2 * TCW],
                          in_=xT[:, 0:hnd, TCW:2 * TCW])
        nc.scalar.dma_start(out=xsb[:, hnd:ND, TCW:2 * TCW],
                            in_=xT[:, hnd:ND, TCW:2 * TCW])
    nc.scalar.dma_start(out=cosr_t[:], in_=cosr[:, :])
    nc.scalar.dma_start(out=sins_t[:], in_=sins[:, :])
    nc.scalar.dma_start(out=perm_t[:], in_=permM[:, :])
    nc.sync.dma_start(out=wvsb[:], in_=wvT[:, :, :])
    for c0 in range(2 * TCW, T, TCW):
        nc.sync.dma_start(out=xsb[:, 0:hnd, c0:c0 + TCW],
                          in_=xT[:, 0:hnd, c0:c0 + TCW])
        nc.scalar.dma_start(out=xsb[:, hnd:ND, c0:c0 + TCW],
                            in_=xT[:, hnd:ND, c0:c0 + TCW])
    nc.sync.dma_start(out=wqsb[:, :, 128:512], in_=wqT[:, :, 128:512])

    triu = pers.tile([128, 128], BF16, tag="triu")
    make_upper_triangular(nc, triu[:], val=1.0, diag=True)

    # V' tiles: [kv0 64 | one | kv1 64 | one]
    vts = []
    for ti in range(NT):
        v = pers.tile([128, 130], BF16, tag=f"v{ti}", name=f"v{ti}")
        nc.vector.memset(v[:, 64:65], 1.0)
        nc.vector.memset(v[:, 129:130], 1.0)
        vts.append(v)

    kt = pers.tile([128, T], BF16, tag="kt")
    qtile = {}

    def get_qt(m):
        if m not in qtile:
            qtile[m] = qpool.tile([128, T], BF16, tag="qt", name=f"qt{m}")
        return qtile[m]

    def rope(pool, tgt, c0, cw):
        """RoPE on tgt[:, c0:c0+cw]: partition swap via perm matmul on PE,
        then 3 DVE passes (sign is folded into sins host-side)."""
        pp = pool.tile([128, cw], F32, tag="pj", name="ropeps")
        nc.tensor.matmul(pp[:], perm_t[:], tgt[:, c0:c0 + cw],
                         start=True, stop=True)
        swp = rp.tile([128, cw], BF16, tag="swp", name="swp")
        nc.vector.tensor_tensor(out=swp[:], in0=pp[:],
                                in1=sins_t[:, c0:c0 + cw], op=ALU.mult)
        tmp = rp.tile([128, cw], BF16, tag="tmp", name="tmp")
        nc.vector.tensor_tensor(out=tmp[:], in0=tgt[:, c0:c0 + cw],
                                in1=cosr_t[:, c0:c0 + cw], op=ALU.mult)
        nc.vector.tensor_tensor(out=tgt[:, c0:c0 + cw], in0=tmp[:], in1=swp[:],
                                op=ALU.add)

    def kq_quanta(pool, m, tcI, copy_eng):
        """4 quanta for one t-chunk of the K (m is None) or Qm projection."""
        state = {}

        def quantum(k):
            if k == 0:
                state["ps"] = pool.tile([128, TCW], F32, tag="pj", name="pjq")
            ps = state["ps"]
            for di in range(k * 4, k * 4 + 4):
                lhs = wkTs[di][:] if m is None else \
                    wqTs[di][:, m * 128:(m + 1) * 128]
                nc.tensor.matmul(
                    ps[:], lhs, xTs[di][:, tcI * TCW:(tcI + 1) * TCW],
                    start=(di == 0), stop=(di == ND - 1))
            if k == 3:
                tgt = kt if m is None else get_qt(m)
                copy_eng(tgt[:, tcI * TCW:(tcI + 1) * TCW], ps[:])

        return [lambda kk=k: quantum(kk) for k in range(4)]

    def v_filler(pool, ti, copy_eng):
        def f():
            psf = pool.tile([128, TCW], F32, tag="pj", name="pjv")
            ps = psf[:, 0:128]
            for di in range(ND):
                nc.tensor.matmul(
                    ps, xTs[di][:, ti * 128:(ti + 1) * 128], wvTs[di][:],
                    start=(di == 0), stop=(di == ND - 1))
            copy_eng(vts[ti][:, 0:64], ps[:, 0:64])
            copy_eng(vts[ti][:, 65:129], ps[:, 64:128])
        return f

    # ---- phase A: PE warm-up, K c0, Q0 c0-c1 + ropes (everything else
    # is band fillers).  Dummy triu matmuls pad the DMA-wait points. ----
    nchunks_A = max(1, HT // TCW)
    with tc.tile_pool(name="ppe", bufs=4, space="PSUM") as ppe:
        warm = ppe.tile([128, 128], F32, tag="warm", bufs=1, name="warm")

        def dummyA(n):
            for _ in range(n):
                nc.tensor.matmul(warm[:], triu[:], triu[:],
                                 start=True, stop=True)

        dummyA(WARMUP_MMS)
        vcopy = nc.vector.tensor_copy
        for q in kq_quanta(ppe, None, 0, vcopy):
            q()
        for tcI in range(nchunks_A):
            if tcI:
                dummyA(26)
            for q in kq_quanta(ppe, 0, tcI, vcopy):
                q()
        rope(ppe, kt, 0, TCW)
        for tcI in range(nchunks_A):
            rope(ppe, get_qt(0), tcI * TCW, TCW)

    # ---- attention pools (8 PSUM banks: 5 + 1 + 2) ----
    sp = ctx.enter_context(tc.tile_pool(name="spsum", bufs=5, space="PSUM"))
    pvp = ctx.enter_context(tc.tile_pool(name="pvp", bufs=1, space="PSUM"))
    projp = ctx.enter_context(tc.tile_pool(name="projp", bufs=2, space="PSUM"))

    # ---- filler schedule: EDF into global bands with per-band budgets ----
    def binfo(g):
        for bi, (m, qlo, qhi) in enumerate(BLOCKS):
            n = qhi // 128
            if g < block_base[bi] + n:
                return bi, m, qlo, qhi, g - block_base[bi]
        raise AssertionError

    def committed(g):
        _, _, qlo, qhi, ki = binfo(g)
        c = (qhi - max(128 * ki, qlo)) / 2400.0
        wqt = ki - LAG
        if wqt >= qlo // 128:
            c += (wqt + 1) * 2 * 0.035
        return c

    def cap(g):
        return 3.2 if g < block_base[2] else 2.2

    scopy = nc.scalar.copy
    groups = []  # (deadline_band, [(cost_us, closure)])
    for tcI in range(1, NTC):
        # kt tile 4*tcI first read at the band with that k-tile index
        t0 = tcI * (TCW // 128)
        dl = t0 if t0 < HT // 128 else block_base[1] + t0
        items = [(0.9, q) for q in kq_quanta(projp, None, tcI, scopy)]
        items.append((0.3, (lambda tcI=tcI: rope(projp, kt, tcI * TCW, TCW))))
        groups.append((dl, items))
    for tcI in range(nchunks_A, NTC):
        # Q0 hi columns: needed by the first band of block (0, HT, T)
        items = [(0.9, q) for q in kq_quanta(projp, 0, tcI, scopy)]
        items.append((0.3, (lambda tcI=tcI: rope(projp, get_qt(0),
                                                 tcI * TCW, TCW))))
        groups.append((block_base[1], items))
    for ti in range(NT):
        # V tile ti: read by PV group qt=ti, woven at band ti+LAG of the
        # first block containing qt=ti
        if ti < HT // 128:
            dl = min(ti + LAG, HT // 128 - 1)
            if ti + LAG > HT // 128 - 1:
                dl = block_base[1]
        else:
            dl = block_base[1] + min(ti + LAG, NT - 1)
        groups.append((dl, [(0.95, v_filler(projp, ti, scopy))]))
    for m in (1, 2, 3):
        for tcI in range(NTC):
            items = [(0.9, q) for q in kq_quanta(projp, m, tcI, scopy)]
            items.append(
                (0.3, (lambda m=m, tcI=tcI: rope(projp, get_qt(m),
                                                 tcI * TCW, TCW))))
            groups.append((block_base[m + 1], items))

    sched = {}
    sched_cost = {}
    groups.sort(key=lambda x: x[0])
    g, used = 0, 0.0
    for dl, items in groups:
        for cost, fn in items:
            budget = max(0.4, cap(g) - committed(g))
            if used >= budget:
                g, used = g + 1, 0.0
                assert g < NBANDS, "filler overflow"
            assert g < dl, f"filler deadline violated: band {g} >= {dl}"
            sched.setdefault(g, []).append(fn)
            sched_cost[g] = sched_cost.get(g, 0.0) + cost
            used += cost

    # ---- attention blocks: bands with S lookahead, woven PV, per-group
    # ACT/DVE copy, keep-warm dummies ----
    pvstate = {"bank": None}
    stgs = {}

    def get_stg(m, half):
        key = (m, half)
        if key not in stgs:
            stgs[key] = stgp.tile([128, NT // 2, 130], F32, tag=f"stg{half}",
                                  name=f"stg{half}_{m}")
        return stgs[key]

    def block(bi):
        m, qlo, qhi = BLOCKS[bi]
        gbase = block_base[bi]
        top = qhi // 128 - 1
        qt0 = qlo // 128
        qtm = get_qt(m)
        ptsAs, ptsBs = {}, {}

        def emit_group(qt):
            # the two head-regions of one qt sit adjacently (130 wide) in
            # the PSUM bank ring so a single copy drains both
            if qt % 2 == 0 or pvstate["bank"] is None:
                pvstate["bank"] = pvp.tile([128, 512], F32, tag="pv",
                                           name="pv")
            bank = pvstate["bank"]
            base = 130 * (qt % 2)
            for h, pts_d in ((0, ptsAs), (1, ptsBs)):
                col = base + 65 * h
                for ki in range(qt + 1):
                    q0 = max(128 * ki, qlo)
                    nc.tensor.matmul(
                        bank[:, col:col + 65],
                        pts_d[ki][:, 128 * qt - q0:128 * qt - q0 + 128],
                        vts[ki][:, h * 65:(h + 1) * 65],
                        start=(ki == 0), stop=(ki == qt))
            half = 0 if qt < NT // 2 else 1
            stg = get_stg(m, half)
            j = qt - half * (NT // 2)
            cp = nc.scalar.copy if qt % 2 == 0 else nc.vector.tensor_copy
            cp(stg[:, j, :], bank[:, base:base + 130])
            if qt == NT // 2 - 1:
                nc.sync.dma_start(
                    out=out[:, 0:NT // 2, m * 130:(m + 1) * 130],
                    in_=get_stg(m, 0)[:])
            elif qt == (3 * NT) // 4 - 1:
                nc.sync.dma_start(
                    out=out[:, NT // 2:(3 * NT) // 4,
                            m * 130:(m + 1) * 130],
                    in_=get_stg(m, 1)[:, 0:(3 * NT) // 4 - NT // 2, :])
            elif qt == NT - 1:
                nc.sync.dma_start(
                    out=out[:, (3 * NT) // 4:NT, m * 130:(m + 1) * 130],
                    in_=get_stg(m, 1)[:, (3 * NT) // 4 - NT // 2:, :])

        def emit_S(ki):
            q0 = max(128 * ki, qlo)
            w = qhi - q0
            ptsA = ppts.tile([128, w], BF16, tag=f"pa{ki}", name=f"pa{ki}")
            ptsB = ppts.tile([128, w], BF16, tag=f"pb{ki}", name=f"pb{ki}")
            for off in range(0, w, SCW):
                cn = min(SCW, w - off)
                sA = sp.tile([128, SCW], F32, tag="s", name="sA")
                sB = sp.tile([128, SCW], F32, tag="s", name="sB")
                nc.tensor.matmul(
                    sA[:, 0:cn],
                    kt[0:64, ki * 128:(ki + 1) * 128],
                    qtm[0:64, q0 + off:q0 + off + cn],
                    start=True, stop=True)
                nc.tensor.matmul(
                    sB[:, 0:cn],
                    kt[64:128, ki * 128:(ki + 1) * 128],
                    qtm[64:128, q0 + off:q0 + off + cn],
                    start=True, stop=True)
                nc.scalar.activation(ptsA[:, off:off + cn], sA[:, 0:cn],
                                     AF.Exp, scale=SCALE)
                nc.vector.tensor_scalar(
                    out=ptsB[:, off:off + cn].bitcast(I16), in0=sB[:, 0:cn],
                    scalar1=EXP_A, scalar2=EXP_B, op0=ALU.mult, op1=ALU.add)
            if 128 * ki >= qlo:  # band contains the diagonal block
                nc.gpsimd.tensor_tensor(out=ptsA[:, 0:128],
                                        in0=ptsA[:, 0:128], in1=triu[:],
                                        op=ALU.mult)
                nc.gpsimd.tensor_tensor(out=ptsB[:, 0:128],
                                        in0=ptsB[:, 0:128], in1=triu[:],
                                        op=ALU.mult)
            ptsAs[ki], ptsBs[ki] = ptsA, ptsB

        next_s = 0
        for ki in range(top + 1):
            # S lookahead: one extra band once bands are narrow, so the
            # exp engines always have queued work while PE runs PV
            look = ki + 1 if ki + 1 <= top and \
                (qhi - 128 * (ki + 1)) <= 2 * SCW and ki >= 2 else ki
            while next_s <= min(look, top):
                emit_S(next_s)
                next_s += 1

            for f in sched.get(gbase + ki, ()):
                f()

            if ki - LAG >= qt0:
                emit_group(ki - LAG)

            # keep-warm: pad underloaded bands with dep-free matmuls into
            # the unused tail of the current PV bank
            if pvstate["bank"] is not None:
                load = committed(gbase + ki) + sched_cost.get(gbase + ki,
                                                              0.0)
                # pad toward the exp slot time so the HAM clock gate stays
                # warm through exp-bound bands
                wband = qhi - max(128 * ki, qlo)
                target = max(1.4, 0.85 * wband * 1.35e-3)
                ndum = min(10, max(0, int((target - load) / 0.107)))
                for _ in range(ndum):
                    nc.tensor.matmul(pvstate["bank"][:, 260:512],
                                     triu[:], kt[:, 0:252],
                                     start=True, stop=True)

        for qt in range(max(qt0, top + 1 - LAG), top + 1):
            emit_group(qt)

    for bi in range(len(BLOCKS)):
        block(bi)

    ctx.close()


def build_program(T=2048, num_devices=8):
    nc = bacc.Bacc("TRN2", target_bir_lowering=False, debug=False,
                   num_devices=num_devices)
    nd = D // 128
    xT = nc.dram_tensor("xT", (128, nd, T), BF16, kind="ExternalInput").ap()
    wqT = nc.dram_tensor("wqT", (128, nd, 512), BF16, kind="ExternalInput").ap()
    wkT = nc.dram_tensor("wkT", (128, nd, 128), BF16, kind="ExternalInput").ap()
    wvT = nc.dram_tensor("wvT", (128, nd, 128), BF16, kind="ExternalInput").ap()
    cosr = nc.dram_tensor("cosr", (128, T), BF16, kind="ExternalInput").ap()
    sins = nc.dram_tensor("sins", (128, T), BF16, kind="ExternalInput").ap()
    permM = nc.dram_tensor("perm", (128, 128), BF16, kind="ExternalInput").ap()
    # out[p, qt, c]: row qt*128+p of the logical [T, 520] output; columns
    # pair-major: pair m, head h (0=m, 1=m+4) numerator at
    # c in [130m+65h, +64), denominator at 130m+65h+64
    out = nc.dram_tensor("out", (128, T // 128, 520), F32,
                         kind="ExternalOutput").ap()
    with tile.TileContext(nc) as tc:
        _emit_body(tc, (xT, wqT, wkT, wvT, cosr, sins, permM, out), T)
    nc.compile()
    return nc


# ---------------- host side ----------------

def _qperm(j):
    rows = []
    for m in range(4):
        for r in range(128):
            h = m if r < 64 else m + 4
            d = 2 * (r % 32) + (1 if (r % 64) >= 32 else 0)
            rows.append((8 * j + h) * 64 + d)
    return np.array(rows)


def _kperm(j):
    rows = []
    for kv in range(2):
        for r in range(64):
            d = 2 * (r % 32) + (1 if r >= 32 else 0)
            rows.append((2 * j + kv) * 64 + d)
    return np.array(rows)


def _to3d(a):
    """[D, C] -> [128, D//128, C] (partition-major di stacking)."""
    Dd, C = a.shape
    return np.ascontiguousarray(a.reshape(Dd // 128, 128, C).transpose(1, 0, 2))


def _perm_mat():
    p = np.zeros((128, 128), dtype=ml_dtypes.bfloat16)
    for i in range(128):
        j = i + 32 if (i % 64) < 32 else i - 32
        p[i, j] = 1.0
    return p


def make_core_inputs(x, Wq, Wk, Wv, cos, sin):
    """Per-core input dicts (host prep). x: [B,T,D]."""
    bf = ml_dtypes.bfloat16
    B, T, _ = x.shape
    xTb = [_to3d(np.ascontiguousarray(x[b].T).astype(bf)) for b in range(B)]
    cosT = np.ascontiguousarray(cos.T.astype(np.float32))  # [32, T]
    sinT = np.ascontiguousarray(sin.T.astype(np.float32))
    cosr = np.tile(cosT, (4, 1)).astype(bf)
    sgn = np.repeat(np.array([-1.0, 1.0, -1.0, 1.0], np.float32), 32)
    sins = (np.tile(sinT, (4, 1)) * sgn[:, None]).astype(bf)
    perm = _perm_mat()
    maps = []
    for c in range(8):
        b, j = c // 4, c % 4
        maps.append({
            "xT": xTb[b],
            "wqT": _to3d(Wq[_qperm(j)].T.astype(bf)),
            "wkT": _to3d(Wk[_kperm(j)].T.astype(bf)),
            "wvT": _to3d(Wv[128 * j:128 * (j + 1)].T.astype(bf)),
            "cosr": cosr,
            "sins": sins,
            "perm": perm,
        })
    return maps


def core_out_to_full(res_out):
    """res_out: [128, NT, 520] pair-major num/den -> [T, 512] head-major."""
    nt = res_out.shape[1]
    o = np.transpose(res_out, (1, 0, 2)).reshape(nt * 128, 520)
    full = np.empty((nt * 128, 512), np.float32)
    for m in range(4):
        for h in (0, 1):
            base = m * 130 + h * 65
            num = o[:, base:base + 64]
            den = o[:, base + 64:base + 65]
            full[:, (m + 4 * h) * 64:(m + 4 * h) * 64 + 64] = num / den
    return full


_CACHE = {}


def _get_program():
    if "nc" not in _CACHE:
        _CACHE["nc"] = build_program(T=2048, num_devices=8)
    return _CACHE["nc"]


def run_on_hw(in_maps, trace=False):
    nc = _get_program()
    return run_bass_kernel_spmd(nc, in_maps, list(range(8)), trace=trace)


def kernel(x, Wq, Wk, Wv, cos, sin):
    x = np.asarray(x, np.float32)
    Wq = np.asarray(Wq, np.float32)
    Wk = np.asarray(Wk, np.float32)
    Wv = np.asarray(Wv, np.float32)
    cos = np.asarray(cos, np.float32)
    sin = np.asarray(sin, np.float32)
    maps = make_core_inputs(x, Wq, Wk, Wv, cos, sin)
    res = run_on_hw(maps, trace=False)
    B, T = x.shape[0], x.shape[1]
    out = np.empty((B, T, 2048), np.float32)
    for c in range(8):
        b, j = c // 4, c % 4
        out[b, :, 512 * j:512 * (j + 1)] = core_out_to_full(res.results[c]["out"])
    return out


# revision 11
# speedup vs baseline: 1.0221x; 1.0221x over previous
"""GQA kernel v4 for Trainium2 (Bass/Tile), 8 NeuronCores.

Sharding: core c -> batch b=c//4, kv-head pair j=c%4 (kv heads 2j,2j+1,
q heads 8j..8j+7).  Each core computes out[b, :, 512j:512(j+1)] (pair-major
column order + per-head softmax denominator; divided + untangled on host).

v4 structure (trace-driven, from the 288us baseline):
  - ONE attention block per head-pair (q-block = full T): bands ki=0..15,
    each band is S^T(ki) over q cols [128ki, T).  PV accumulation group
    for q-tile qt is woven right after band qt+LAG, so PV matmuls spread
    across the whole pair instead of bursting at block end (the v3
    qb-split left 10-band "deserts" with no PE work -> HAM half-clock).
  - PV regions live in a ring of [128,512] PSUM bank tiles (7 x 65-wide
    regions each, bufs=2); each group's [128,65] numerator+denominator
    is copied to SBUF by GPSIMD right after the group closes (ACT/DVE
    stay free for exp; no end-of-block copy chain).
  - S PSUM ring (tag "s", bufs=4, 512-wide chunks) decouples S matmuls
    from exp consumption.
  - Input DMA split across the two HWDGE queues (sync + scalar),
    priority-ordered so phase A streams just-in-time.
  - PE warm-up matmuls at t=0 (triu x triu) warm the HAM clock gate.
  - RoPE partition swap via a signed-permutation matmul on the PE.
  - Softmax division on the host (numerators + ones-column denominator).
  - exp split per band: head m via ACT (exact), head m+4 via DVE int16
    Schraudolph writing bf16 bit patterns.
"""

import sys

for _p in ("/opt/trn_rl_repo",):
    if _p not in sys.path:
        sys.path.insert(0, _p)

import contextlib

import numpy as np
import ml_dtypes

import concourse.bass as bass
import concourse.tile as tile
from concourse import bacc, mybir
from concourse.bass_utils import run_bass_kernel_spmd
from concourse.masks import make_upper_triangular

BF16 = mybir.dt.bfloat16
F32 = mybir.dt.float32
I16 = mybir.dt.int16
AF = mybir.ActivationFunctionType
ALU = mybir.AluOpType

D = 2048
HS = 64
SCALE = 1.0 / 8.0  # 1/sqrt(HS)

EXP_A = float(128.0 * 1.4426950408889634 * SCALE)
EXP_B = float(127.0 * 128.0 - 366393.0 / 65536.0 - 0.5)

WARMUP_MMS = 44
LAG = 2


def _emit_body(tc, aps, T):
    nc = tc.nc
    NT = T // 128            # k/q tiles
    ND = D // 128            # contraction chunks
    TCW = min(512, T)        # projection t-chunk width
    NTC = T // TCW
    SCW = min(512, T)        # S-chunk width
    HT = T // 2

    BLOCKS = [(0, 0, T), (1, 0, T), (2, 0, T), (3, 0, T)]
    block_base = {}
    _g = 0
    for bi, (m, qlo, qhi) in enumerate(BLOCKS):
        block_base[bi] = _g
        _g += qhi // 128
    NBANDS = _g

    xT, wqT, wkT, wvT, cosr, sins, permM, out = aps

    ctx = tc._kernel_exitstack = contextlib.ExitStack()

    pers = ctx.enter_context(tc.tile_pool(name="pers", bufs=1))
    rp = ctx.enter_context(tc.tile_pool(name="rope", bufs=2))
    ppts = ctx.enter_context(tc.tile_pool(name="ppts", bufs=1))
    stgp = ctx.enter_context(tc.tile_pool(name="stgp", bufs=1))
    qpool = ctx.enter_context(tc.tile_pool(name="qpool", bufs=2))

    # ---- persistent tiles ----
    xsb = pers.tile([128, ND, T], BF16, tag="xsb")
    wqsb = pers.tile([128, ND, 512], BF16, tag="wqsb")
    wksb = pers.tile([128, ND, 128], BF16, tag="wksb")
    wvsb = pers.tile([128, ND, 128], BF16, tag="wvsb")
    xTs = [xsb[:, di, :] for di in range(ND)]
    wqTs = [wqsb[:, di, :] for di in range(ND)]
    wkTs = [wksb[:, di, :] for di in range(ND)]
    wvTs = [wvsb[:, di, :] for di in range(ND)]
    cosr_t = pers.tile([128, T], BF16, tag="cosr")
    sins_t = pers.tile([128, T], BF16, tag="sins")
    perm_t = pers.tile([128, 128], BF16, tag="perm")

    # ---- input DMA: two HWDGE queues, ordered by first-use time ----
    hnd = ND // 2
    qnd = ND // 4
    nc.scalar.dma_start(out=wksb[:], in_=wkT[:, :, :])
    nc.sync.dma_start(out=xsb[:, 0:qnd, 0:TCW], in_=xT[:, 0:qnd, 0:TCW])
    nc.sync.dma_start(out=xsb[:, qnd:hnd, 0:TCW], in_=xT[:, qnd:hnd, 0:TCW])
    nc.scalar.dma_start(out=xsb[:, hnd:3 * qnd, 0:TCW],
                        in_=xT[:, hnd:3 * qnd, 0:TCW])
    nc.scalar.dma_start(out=xsb[:, 3 * qnd:ND, 0:TCW],
                        in_=xT[:, 3 * qnd:ND, 0:TCW])
    nc.sync.dma_start(out=wqsb[:, :, 0:128], in_=wqT[:, :, 0:128])
    if T > TCW:
        nc.sync.dma_start(out=xsb[:, 0:hnd, TCW:2 * TCW],
                          in_=xT[:, 0:hnd, TCW:2 * TCW])
        nc.scalar.dma_start(out=xsb[:, hnd:ND, TCW:2 * TCW],
                            in_=xT[:, hnd:ND, TCW:2 * TCW])
    nc.scalar.dma_start(out=cosr_t[:], in_=cosr[:, :])
    nc.scalar.dma_start(out=sins_t[:], in_=sins[:, :])
    nc.scalar.dma_start(out=perm_t[:], in_=permM[:, :])
    nc.sync.dma_start(out=wvsb[:], in_=wvT[:, :, :])
    for c0 in range(2 * TCW, T, TCW):
        nc.sync.dma_start(out=xsb[:, 0:hnd, c0:c0 + TCW],
                          in_=xT[:, 0:hnd, c0:c0 + TCW])
        nc.scalar.dma_start(out=xsb[:, hnd:ND, c0:c0 + TCW],
                            in_=xT[:, hnd:ND, c0:c0 + TCW])
    nc.sync.dma_start(out=wqsb[:, :, 128:512], in_=wqT[:, :, 128:512])

    triu = pers.tile([128, 128], BF16, tag="triu")
    make_upper_triangular(nc, triu[:], val=1.0, diag=True)

    # V' tiles: [kv0 64 | one | kv1 64 | one]
    vts = []
    for ti in range(NT):
        v = pers.tile([128, 130], BF16, tag=f"v{ti}", name=f"v{ti}")
        nc.vector.memset(v[:, 64:65], 1.0)
        nc.vector.memset(v[:, 129:130], 1.0)
        vts.append(v)

    kt = pers.tile([128, T], BF16, tag="kt")
    qtile = {}

    def get_qt(m):
        if m not in qtile:
            qtile[m] = qpool.tile([128, T], BF16, tag="qt", name=f"qt{m}")
        return qtile[m]

    def rope(pool, tgt, c0, cw):
        """RoPE on tgt[:, c0:c0+cw]: partition swap via perm matmul on PE,
        then 3 DVE passes (sign is folded into sins host-side)."""
        pp = pool.tile([128, cw], F32, tag="pj", name="ropeps")
        nc.tensor.matmul(pp[:], perm_t[:], tgt[:, c0:c0 + cw],
                         start=True, stop=True)
        swp = rp.tile([128, cw], BF16, tag="swp", name="swp")
        nc.vector.tensor_tensor(out=swp[:], in0=pp[:],
                                in1=sins_t[:, c0:c0 + cw], op=ALU.mult)
        tmp = rp.tile([128, cw], BF16, tag="tmp", name="tmp")
        nc.vector.tensor_tensor(out=tmp[:], in0=tgt[:, c0:c0 + cw],
                                in1=cosr_t[:, c0:c0 + cw], op=ALU.mult)
        nc.vector.tensor_tensor(out=tgt[:, c0:c0 + cw], in0=tmp[:], in1=swp[:],
                                op=ALU.add)

    def kq_quanta(pool, m, tcI, copy_eng):
        """4 quanta for one t-chunk of the K (m is None) or Qm projection."""
        state = {}

        def quantum(k):
            if k == 0:
                state["ps"] = pool.tile([128, TCW], F32, tag="pj", name="pjq")
            ps = state["ps"]
            for di in range(k * 4, k * 4 + 4):
                lhs = wkTs[di][:] if m is None else \
                    wqTs[di][:, m * 128:(m + 1) * 128]
                nc.tensor.matmul(
                    ps[:], lhs, xTs[di][:, tcI * TCW:(tcI + 1) * TCW],
                    start=(di == 0), stop=(di == ND - 1))
            if k == 3:
                tgt = kt if m is None else get_qt(m)
                copy_eng(tgt[:, tcI * TCW:(tcI + 1) * TCW], ps[:])

        return [lambda kk=k: quantum(kk) for k in range(4)]

    def v_filler(pool, ti, copy_eng):
        def f():
            psf = pool.tile([128, TCW], F32, tag="pj", name="pjv")
            ps = psf[:, 0:128]
            for di in range(ND):
                nc.tensor.matmul(
                    ps, xTs[di][:, ti * 128:(ti + 1) * 128], wvTs[di][:],
                    start=(di == 0), stop=(di == ND - 1))
            copy_eng(vts[ti][:, 0:64], ps[:, 0:64])
            copy_eng(vts[ti][:, 65:129], ps[:, 64:128])
        return f

    # ---- phase A: PE warm-up, K c0, Q0 c0-c1 + ropes (everything else
    # is band fillers).  Dummy triu matmuls pad the DMA-wait points. ----
    nchunks_A = NTC
    with tc.tile_pool(name="ppe", bufs=4, space="PSUM") as ppe:
        warm = ppe.tile([128, 128], F32, tag="warm", bufs=1, name="warm")

        def dummyA(n):
            for _ in range(n):
                nc.tensor.matmul(warm[:], triu[:], triu[:],
                                 start=True, stop=True)

        dummyA(WARMUP_MMS)
        vcopy = nc.vector.tensor_copy
        for q in kq_quanta(ppe, None, 0, vcopy):
            q()
        for tcI in range(nchunks_A):
            if tcI:
                dummyA(16)
            if tcI == 2:
                v_filler(ppe, 0, vcopy)()
                v_filler(ppe, 1, vcopy)()
            if tcI == NTC - 1:
                v_filler(ppe, 2, vcopy)()
                v_filler(ppe, 3, vcopy)()
            for q in kq_quanta(ppe, 0, tcI, vcopy):
                q()
            if tcI == 0:
                rope(ppe, kt, 0, TCW)
            rope(ppe, get_qt(0), tcI * TCW, TCW)

    # ---- attention pools (8 PSUM banks: 5 + 1 + 2) ----
    sp = ctx.enter_context(tc.tile_pool(name="spsum", bufs=5, space="PSUM"))
    pvp = ctx.enter_context(tc.tile_pool(name="pvp", bufs=1, space="PSUM"))
    projp = ctx.enter_context(tc.tile_pool(name="projp", bufs=2, space="PSUM"))

    # ---- filler schedule: EDF into global bands with per-band budgets ----
    def binfo(g):
        for bi, (m, qlo, qhi) in enumerate(BLOCKS):
            n = qhi // 128
            if g < block_base[bi] + n:
                return bi, m, qlo, qhi, g - block_base[bi]
        raise AssertionError

    def committed(g):
        _, _, qlo, qhi, ki = binfo(g)
        c = (qhi - max(128 * ki, qlo)) / 2400.0
        wqt = ki - LAG
        if wqt >= qlo // 128:
            c += (wqt + 1) * 2 * 0.035
        return c

    def cap(g):
        return 3.4 if g < block_base[1] else 2.2

    scopy = nc.scalar.copy
    groups = []  # (deadline_band, [(cost_us, closure)])
    for tcI in range(1, NTC):
        # kt tile 4*tcI first read at the band with that k-tile index
        dl = tcI * (TCW // 128)
        items = [(0.9, q) for q in kq_quanta(projp, None, tcI, scopy)]
        items.append((0.3, (lambda tcI=tcI: rope(projp, kt, tcI * TCW, TCW))))
        groups.append((dl, items))
    for ti in range(4, NT):
        # V tile ti: read by PV group qt=ti, woven at band ti+LAG
        groups.append((min(ti + LAG, NT - 1),
                       [(0.95, v_filler(projp, ti, scopy))]))
    for m in (1, 2, 3):
        for tcI in range(NTC):
            items = [(0.9, q) for q in kq_quanta(projp, m, tcI, scopy)]
            items.append(
                (0.3, (lambda m=m, tcI=tcI: rope(projp, get_qt(m),
                                                 tcI * TCW, TCW))))
            groups.append((block_base[m], items))

    sched = {}
    sched_cost = {}
    groups.sort(key=lambda x: x[0])
    g, used = 0, 0.0
    for dl, items in groups:
        for cost, fn in items:
            budget = max(0.4, cap(g) - committed(g))
            if used >= budget:
                g, used = g + 1, 0.0
                assert g < NBANDS, "filler overflow"
            assert g < dl, f"filler deadline violated: band {g} >= {dl}"
            sched.setdefault(g, []).append(fn)
            sched_cost[g] = sched_cost.get(g, 0.0) + cost
            used += cost

    # ---- attention blocks: bands with S lookahead, woven PV, per-group
    # ACT/DVE copy, keep-warm dummies ----
    pvstate = {"bank": None}
    stgs = {}

    def get_stg(m, half):
        key = (m, half)
        if key not in stgs:
            stgs[key] = stgp.tile([128, NT // 2, 130], F32, tag=f"stg{half}",
                                  name=f"stg{half}_{m}")
        return stgs[key]

    def block(bi):
        m, qlo, qhi = BLOCKS[bi]
        gbase = block_base[bi]
        top = qhi // 128 - 1
        qt0 = qlo // 128
        qtm = get_qt(m)
        ptsAs, ptsBs = {}, {}

        def emit_group(qt):
            # the two head-regions of one qt sit adjacently (130 wide) in
            # the PSUM bank ring so a single copy drains both
            if qt % 2 == 0 or pvstate["bank"] is None:
                pvstate["bank"] = pvp.tile([128, 512], F32, tag="pv",
                                           name="pv")
            bank = pvstate["bank"]
            base = 130 * (qt % 2)
            for h, pts_d in ((0, ptsAs), (1, ptsBs)):
                col = base + 65 * h
                for ki in range(qt + 1):
                    q0 = max(128 * ki, qlo)
                    nc.tensor.matmul(
                        bank[:, col:col + 65],
                        pts_d[ki][:, 128 * qt - q0:128 * qt - q0 + 128],
                        vts[ki][:, h * 65:(h + 1) * 65],
                        start=(ki == 0), stop=(ki == qt))
            half = 0 if qt < NT // 2 else 1
            stg = get_stg(m, half)
            j = qt - half * (NT // 2)
            cp = nc.scalar.copy if qt % 2 == 0 else nc.vector.tensor_copy
            cp(stg[:, j, :], bank[:, base:base + 130])
            if qt == NT // 2 - 1:
                nc.sync.dma_start(
                    out=out[:, 0:NT // 2, m * 130:(m + 1) * 130],
                    in_=get_stg(m, 0)[:])
            elif qt == (3 * NT) // 4 - 1:
                nc.sync.dma_start(
                    out=out[:, NT // 2:(3 * NT) // 4,
                            m * 130:(m + 1) * 130],
                    in_=get_stg(m, 1)[:, 0:(3 * NT) // 4 - NT // 2, :])
            elif qt == NT - 1:
                nc.sync.dma_start(
                    out=out[:, (3 * NT) // 4:NT, m * 130:(m + 1) * 130],
                    in_=get_stg(m, 1)[:, (3 * NT) // 4 - NT // 2:, :])

        def emit_S(ki):
            q0 = max(128 * ki, qlo)
            w = qhi - q0
            ptsA = ppts.tile([128, w], BF16, tag=f"pa{ki}", name=f"pa{ki}")
            ptsB = ppts.tile([128, w], BF16, tag=f"pb{ki}", name=f"pb{ki}")
            for off in range(0, w, SCW):
                cn = min(SCW, w - off)
                sA = sp.tile([128, SCW], F32, tag="s", name="sA")
                sB = sp.tile([128, SCW], F32, tag="s", name="sB")
                nc.tensor.matmul(
                    sA[:, 0:cn],
                    kt[0:64, ki * 128:(ki + 1) * 128],
                    qtm[0:64, q0 + off:q0 + off + cn],
                    start=True, stop=True)
                nc.tensor.matmul(
                    sB[:, 0:cn],
                    kt[64:128, ki * 128:(ki + 1) * 128],
                    qtm[64:128, q0 + off:q0 + off + cn],
                    start=True, stop=True)
                nc.scalar.activation(ptsA[:, off:off + cn], sA[:, 0:cn],
                                     AF.Exp, scale=SCALE)
                nc.vector.tensor_scalar(
                    out=ptsB[:, off:off + cn].bitcast(I16), in0=sB[:, 0:cn],
                    scalar1=EXP_A, scalar2=EXP_B, op0=ALU.mult, op1=ALU.add)
            if 128 * ki >= qlo:  # band contains the diagonal block
                nc.gpsimd.tensor_tensor(out=ptsA[:, 0:128],
                                        in0=ptsA[:, 0:128], in1=triu[:],
                                        op=ALU.mult)
                nc.gpsimd.tensor_tensor(out=ptsB[:, 0:128],
                                        in0=ptsB[:, 0:128], in1=triu[:],
                                        op=ALU.mult)
            ptsAs[ki], ptsBs[ki] = ptsA, ptsB

        next_s = 0
        for ki in range(top + 1):
            # S lookahead: one extra band once bands are narrow, so the
            # exp engines always have queued work while PE runs PV
            look = ki + 1 if ki + 1 <= top and \
                (qhi - 128 * (ki + 1)) <= 2 * SCW and ki >= 2 else ki
            while next_s <= min(look, top):
                emit_S(next_s)
                next_s += 1

            for f in sched.get(gbase + ki, ()):
                f()

            if ki - LAG >= qt0:
                emit_group(ki - LAG)

            # keep-warm: pad underloaded bands with dep-free matmuls into
            # the unused tail of the current PV bank
            if pvstate["bank"] is not None:
                load = committed(gbase + ki) + sched_cost.get(gbase + ki,
                                                              0.0)
                # pad toward the exp slot time so the HAM clock gate stays
                # warm through exp-bound bands
                wband = qhi - max(128 * ki, qlo)
                target = max(1.4, 0.85 * wband * 1.35e-3)
                ndum = min(10, max(0, int((target - load) / 0.107)))
                for _ in range(ndum):
                    nc.tensor.matmul(pvstate["bank"][:, 260:512],
                                     triu[:], kt[:, 0:252],
                                     start=True, stop=True)

        for qt in range(max(qt0, top + 1 - LAG), top + 1):
            emit_group(qt)

    for bi in range(len(BLOCKS)):
        block(bi)

    ctx.close()


def build_program(T=2048, num_devices=8):
    nc = bacc.Bacc("TRN2", target_bir_lowering=False, debug=False,
                   num_devices=num_devices)
    nd = D // 128
    xT = nc.dram_tensor("xT", (128, nd, T), BF16, kind="ExternalInput").ap()
    wqT = nc.dram_tensor("wqT", (128, nd, 512), BF16, kind="ExternalInput").ap()
    wkT = nc.dram_tensor("wkT", (128, nd, 128), BF16, kind="ExternalInput").ap()
    wvT = nc.dram_tensor("wvT", (128, nd, 128), BF16, kind="ExternalInput").ap()
    cosr = nc.dram_tensor("cosr", (128, T), BF16, kind="ExternalInput").ap()
    sins = nc.dram_tensor("sins", (128, T), BF16, kind="ExternalInput").ap()
    permM = nc.dram_tensor("perm", (128, 128), BF16, kind="ExternalInput").ap()
    # out[p, qt, c]: row qt*128+p of the logical [T, 520] output; columns
    # pair-major: pair m, head h (0=m, 1=m+4) numerator at
    # c in [130m+65h, +64), denominator at 130m+65h+64
    out = nc.dram_tensor("out", (128, T // 128, 520), F32,
                         kind="ExternalOutput").ap()
    with tile.TileContext(nc) as tc:
        _emit_body(tc, (xT, wqT, wkT, wvT, cosr, sins, permM, out), T)
    nc.compile()
    return nc


# ---------------- host side ----------------

def _qperm(j):
    rows = []
    for m in range(4):
        for r in range(128):
            h = m if r < 64 else m + 4
            d = 2 * (r % 32) + (1 if (r % 64) >= 32 else 0)
            rows.append((8 * j + h) * 64 + d)
    return np.array(rows)


def _kperm(j):
    rows = []
    for kv in range(2):
        for r in range(64):
            d = 2 * (r % 32) + (1 if r >= 32 else 0)
            rows.append((2 * j + kv) * 64 + d)
    return np.array(rows)


def _to3d(a):
    """[D, C] -> [128, D//128, C] (partition-major di stacking)."""
    Dd, C = a.shape
    return np.ascontiguousarray(a.reshape(Dd // 128, 128, C).transpose(1, 0, 2))


def _perm_mat():
    p = np.zeros((128, 128), dtype=ml_dtypes.bfloat16)
    for i in range(128):
        j = i + 32 if (i % 64) < 32 else i - 32
        p[i, j] = 1.0
    return p


def make_core_inputs(x, Wq, Wk, Wv, cos, sin):
    """Per-core input dicts (host prep). x: [B,T,D]."""
    bf = ml_dtypes.bfloat16
    B, T, _ = x.shape
    xTb = [_to3d(np.ascontiguousarray(x[b].T).astype(bf)) for b in range(B)]
    cosT = np.ascontiguousarray(cos.T.astype(np.float32))  # [32, T]
    sinT = np.ascontiguousarray(sin.T.astype(np.float32))
    cosr = np.tile(cosT, (4, 1)).astype(bf)
    sgn = np.repeat(np.array([-1.0, 1.0, -1.0, 1.0], np.float32), 32)
    sins = (np.tile(sinT, (4, 1)) * sgn[:, None]).astype(bf)
    perm = _perm_mat()
    maps = []
    for c in range(8):
        b, j = c // 4, c % 4
        maps.append({
            "xT": xTb[b],
            "wqT": _to3d(Wq[_qperm(j)].T.astype(bf)),
            "wkT": _to3d(Wk[_kperm(j)].T.astype(bf)),
            "wvT": _to3d(Wv[128 * j:128 * (j + 1)].T.astype(bf)),
            "cosr": cosr,
            "sins": sins,
            "perm": perm,
        })
    return maps


def core_out_to_full(res_out):
    """res_out: [128, NT, 520] pair-major num/den -> [T, 512] head-major."""
    nt = res_out.shape[1]
    o = np.transpose(res_out, (1, 0, 2)).reshape(nt * 128, 520)
    full = np.empty((nt * 128, 512), np.float32)
    for m in range(4):
        for h in (0, 1):
            base = m * 130 + h * 65
            num = o[:, base:base + 64]
            den = o[:, base + 64:base + 65]
            full[:, (m + 4 * h) * 64:(m + 4 * h) * 64 + 64] = num / den
    return full


_CACHE = {}


def _get_program():
    if "nc" not in _CACHE:
        _CACHE["nc"] = build_program(T=2048, num_devices=8)
    return _CACHE["nc"]


def run_on_hw(in_maps, trace=False):
    nc = _get_program()
    return run_bass_kernel_spmd(nc, in_maps, list(range(8)), trace=trace)


def kernel(x, Wq, Wk, Wv, cos, sin):
    x = np.asarray(x, np.float32)
    Wq = np.asarray(Wq, np.float32)
    Wk = np.asarray(Wk, np.float32)
    Wv = np.asarray(Wv, np.float32)
    cos = np.asarray(cos, np.float32)
    sin = np.asarray(sin, np.float32)
    maps = make_core_inputs(x, Wq, Wk, Wv, cos, sin)
    res = run_on_hw(maps, trace=False)
    B, T = x.shape[0], x.shape[1]
    out = np.empty((B, T, 2048), np.float32)
    for c in range(8):
        b, j = c // 4, c % 4
        out[b, :, 512 * j:512 * (j + 1)] = core_out_to_full(res.results[c]["out"])
    return out


# revision 12
# speedup vs baseline: 1.0575x; 1.0347x over previous
"""GQA kernel v4 for Trainium2 (Bass/Tile), 8 NeuronCores.

Sharding: core c -> batch b=c//4, kv-head pair j=c%4 (kv heads 2j,2j+1,
q heads 8j..8j+7).  Each core computes out[b, :, 512j:512(j+1)] (pair-major
column order + per-head softmax denominator; divided + untangled on host).

v4 structure (trace-driven, from the 288us baseline):
  - ONE attention block per head-pair (q-block = full T): bands ki=0..15,
    each band is S^T(ki) over q cols [128ki, T).  PV accumulation group
    for q-tile qt is woven right after band qt+LAG, so PV matmuls spread
    across the whole pair instead of bursting at block end (the v3
    qb-split left 10-band "deserts" with no PE work -> HAM half-clock).
  - PV regions live in a ring of [128,512] PSUM bank tiles (7 x 65-wide
    regions each, bufs=2); each group's [128,65] numerator+denominator
    is copied to SBUF by GPSIMD right after the group closes (ACT/DVE
    stay free for exp; no end-of-block copy chain).
  - S PSUM ring (tag "s", bufs=4, 512-wide chunks) decouples S matmuls
    from exp consumption.
  - Input DMA split across the two HWDGE queues (sync + scalar),
    priority-ordered so phase A streams just-in-time.
  - PE warm-up matmuls at t=0 (triu x triu) warm the HAM clock gate.
  - RoPE partition swap via a signed-permutation matmul on the PE.
  - Softmax division on the host (numerators + ones-column denominator).
  - exp split per band: head m via ACT (exact), head m+4 via DVE int16
    Schraudolph writing bf16 bit patterns.
"""

import sys

for _p in ("/opt/trn_rl_repo",):
    if _p not in sys.path:
        sys.path.insert(0, _p)

import contextlib

import numpy as np
import ml_dtypes

import concourse.bass as bass
import concourse.tile as tile
from concourse import bacc, mybir
from concourse.bass_utils import run_bass_kernel_spmd
from concourse.masks import make_upper_triangular

BF16 = mybir.dt.bfloat16
F32 = mybir.dt.float32
I16 = mybir.dt.int16
AF = mybir.ActivationFunctionType
ALU = mybir.AluOpType

D = 2048
HS = 64
SCALE = 1.0 / 8.0  # 1/sqrt(HS)

EXP_A = float(128.0 * 1.4426950408889634 * SCALE)
EXP_B = float(127.0 * 128.0 - 366393.0 / 65536.0 - 0.5)

WARMUP_MMS = 44
LAG = 2


def _emit_body(tc, aps, T):
    nc = tc.nc
    NT = T // 128            # k/q tiles
    ND = D // 128            # contraction chunks
    TCW = min(512, T)        # projection t-chunk width
    NTC = T // TCW
    SCW = min(512, T)        # S-chunk width
    HT = T // 2

    BLOCKS = [(0, 0, T), (1, 0, T), (2, 0, T), (3, 0, T)]
    block_base = {}
    _g = 0
    for bi, (m, qlo, qhi) in enumerate(BLOCKS):
        block_base[bi] = _g
        _g += qhi // 128
    NBANDS = _g

    xT, wqT, wkT, wvT, cosr, sins, permM, out = aps

    ctx = tc._kernel_exitstack = contextlib.ExitStack()

    pers = ctx.enter_context(tc.tile_pool(name="pers", bufs=1))
    rp = ctx.enter_context(tc.tile_pool(name="rope", bufs=2))
    ppts = ctx.enter_context(tc.tile_pool(name="ppts", bufs=1))
    stgp = ctx.enter_context(tc.tile_pool(name="stgp", bufs=1))
    qpool = ctx.enter_context(tc.tile_pool(name="qpool", bufs=2))

    # ---- persistent tiles ----
    xsb = pers.tile([128, ND, T], BF16, tag="xsb")
    wqsb = pers.tile([128, ND, 512], BF16, tag="wqsb")
    wksb = pers.tile([128, ND, 128], BF16, tag="wksb")
    wvsb = pers.tile([128, ND, 128], BF16, tag="wvsb")
    xTs = [xsb[:, di, :] for di in range(ND)]
    wqTs = [wqsb[:, di, :] for di in range(ND)]
    wkTs = [wksb[:, di, :] for di in range(ND)]
    wvTs = [wvsb[:, di, :] for di in range(ND)]
    cosr_t = pers.tile([128, T], BF16, tag="cosr")
    sins_t = pers.tile([128, T], BF16, tag="sins")
    perm_t = pers.tile([128, 128], BF16, tag="perm")

    # ---- input DMA: two HWDGE queues, ordered by first-use time ----
    hnd = ND // 2
    qnd = ND // 4
    nc.scalar.dma_start(out=wksb[:], in_=wkT[:, :, :])
    nc.sync.dma_start(out=xsb[:, 0:qnd, 0:TCW], in_=xT[:, 0:qnd, 0:TCW])
    nc.sync.dma_start(out=xsb[:, qnd:hnd, 0:TCW], in_=xT[:, qnd:hnd, 0:TCW])
    nc.scalar.dma_start(out=xsb[:, hnd:3 * qnd, 0:TCW],
                        in_=xT[:, hnd:3 * qnd, 0:TCW])
    nc.scalar.dma_start(out=xsb[:, 3 * qnd:ND, 0:TCW],
                        in_=xT[:, 3 * qnd:ND, 0:TCW])
    nc.sync.dma_start(out=wqsb[:, :, 0:128], in_=wqT[:, :, 0:128])
    if T > TCW:
        nc.sync.dma_start(out=xsb[:, 0:hnd, TCW:2 * TCW],
                          in_=xT[:, 0:hnd, TCW:2 * TCW])
        nc.scalar.dma_start(out=xsb[:, hnd:ND, TCW:2 * TCW],
                            in_=xT[:, hnd:ND, TCW:2 * TCW])
    nc.scalar.dma_start(out=cosr_t[:], in_=cosr[:, :])
    nc.scalar.dma_start(out=sins_t[:], in_=sins[:, :])
    nc.scalar.dma_start(out=perm_t[:], in_=permM[:, :])
    nc.sync.dma_start(out=wvsb[:], in_=wvT[:, :, :])
    for c0 in range(2 * TCW, T, TCW):
        nc.sync.dma_start(out=xsb[:, 0:hnd, c0:c0 + TCW],
                          in_=xT[:, 0:hnd, c0:c0 + TCW])
        nc.scalar.dma_start(out=xsb[:, hnd:ND, c0:c0 + TCW],
                            in_=xT[:, hnd:ND, c0:c0 + TCW])
    nc.sync.dma_start(out=wqsb[:, :, 128:512], in_=wqT[:, :, 128:512])

    triu = pers.tile([128, 128], BF16, tag="triu")
    make_upper_triangular(nc, triu[:], val=1.0, diag=True)

    # V' tiles: [kv0 64 | one | kv1 64 | one]
    vts = []
    for ti in range(NT):
        v = pers.tile([128, 130], BF16, tag=f"v{ti}", name=f"v{ti}")
        nc.vector.memset(v[:, 64:65], 1.0)
        nc.vector.memset(v[:, 129:130], 1.0)
        vts.append(v)

    kt = pers.tile([128, T], BF16, tag="kt")
    qtile = {}

    def get_qt(m):
        if m not in qtile:
            qtile[m] = qpool.tile([128, T], BF16, tag="qt", name=f"qt{m}")
        return qtile[m]

    def rope(pool, tgt, c0, cw):
        """RoPE on tgt[:, c0:c0+cw]: partition swap via perm matmul on PE,
        then 3 DVE passes (sign is folded into sins host-side)."""
        pp = pool.tile([128, cw], F32, tag="pj", name="ropeps")
        nc.tensor.matmul(pp[:], perm_t[:], tgt[:, c0:c0 + cw],
                         start=True, stop=True)
        swp = rp.tile([128, cw], BF16, tag="swp", name="swp")
        nc.vector.tensor_tensor(out=swp[:], in0=pp[:],
                                in1=sins_t[:, c0:c0 + cw], op=ALU.mult)
        tmp = rp.tile([128, cw], BF16, tag="tmp", name="tmp")
        nc.vector.tensor_tensor(out=tmp[:], in0=tgt[:, c0:c0 + cw],
                                in1=cosr_t[:, c0:c0 + cw], op=ALU.mult)
        nc.vector.tensor_tensor(out=tgt[:, c0:c0 + cw], in0=tmp[:], in1=swp[:],
                                op=ALU.add)

    def kq_quanta(pool, m, tcI, copy_eng):
        """4 quanta for one t-chunk of the K (m is None) or Qm projection."""
        state = {}

        def quantum(k):
            if k == 0:
                state["ps"] = pool.tile([128, TCW], F32, tag="pj", name="pjq")
            ps = state["ps"]
            for di in range(k * 4, k * 4 + 4):
                lhs = wkTs[di][:] if m is None else \
                    wqTs[di][:, m * 128:(m + 1) * 128]
                nc.tensor.matmul(
                    ps[:], lhs, xTs[di][:, tcI * TCW:(tcI + 1) * TCW],
                    start=(di == 0), stop=(di == ND - 1))
            if k == 3:
                tgt = kt if m is None else get_qt(m)
                copy_eng(tgt[:, tcI * TCW:(tcI + 1) * TCW], ps[:])

        return [lambda kk=k: quantum(kk) for k in range(4)]

    def v_filler(pool, ti, copy_eng):
        def f():
            psf = pool.tile([128, TCW], F32, tag="pj", name="pjv")
            ps = psf[:, 0:128]
            for di in range(ND):
                nc.tensor.matmul(
                    ps, xTs[di][:, ti * 128:(ti + 1) * 128], wvTs[di][:],
                    start=(di == 0), stop=(di == ND - 1))
            copy_eng(vts[ti][:, 0:64], ps[:, 0:64])
            copy_eng(vts[ti][:, 65:129], ps[:, 64:128])
        return f

    # ---- phase A: PE warm-up, K c0, Q0 c0-c1 + ropes (everything else
    # is band fillers).  Dummy triu matmuls pad the DMA-wait points. ----
    nchunks_A = NTC
    with tc.tile_pool(name="ppe", bufs=4, space="PSUM") as ppe:
        warm = ppe.tile([128, 128], F32, tag="warm", bufs=1, name="warm")

        def dummyA(n):
            for _ in range(n):
                nc.tensor.matmul(warm[:], triu[:], triu[:],
                                 start=True, stop=True)

        dummyA(WARMUP_MMS)
        vcopy = nc.vector.tensor_copy
        for q in kq_quanta(ppe, None, 0, vcopy):
            q()
        for tcI in range(nchunks_A):
            if tcI:
                dummyA(16)
            if tcI == 2:
                v_filler(ppe, 0, vcopy)()
                v_filler(ppe, 1, vcopy)()
            if tcI == NTC - 1:
                v_filler(ppe, 2, vcopy)()
                v_filler(ppe, 3, vcopy)()
            for q in kq_quanta(ppe, 0, tcI, vcopy):
                q()
            if tcI == 0:
                rope(ppe, kt, 0, TCW)
            rope(ppe, get_qt(0), tcI * TCW, TCW)

    # ---- attention pools (8 PSUM banks: 5 + 1 + 2) ----
    sp = ctx.enter_context(tc.tile_pool(name="spsum", bufs=5, space="PSUM"))
    pvp = ctx.enter_context(tc.tile_pool(name="pvp", bufs=1, space="PSUM"))
    projp = ctx.enter_context(tc.tile_pool(name="projp", bufs=2, space="PSUM"))

    # ---- filler schedule: EDF into global bands with per-band budgets ----
    def binfo(g):
        for bi, (m, qlo, qhi) in enumerate(BLOCKS):
            n = qhi // 128
            if g < block_base[bi] + n:
                return bi, m, qlo, qhi, g - block_base[bi]
        raise AssertionError

    def committed(g):
        _, _, qlo, qhi, ki = binfo(g)
        c = (qhi - max(128 * ki, qlo)) / 2400.0
        wqt = ki - LAG
        if wqt >= qlo // 128:
            c += (wqt + 1) * 2 * 0.035
        return c

    def cap(g):
        return 3.4 if g < block_base[1] else 2.2

    scopy = nc.scalar.copy
    groups = []  # (deadline_band, [(cost_us, closure)])
    for tcI in range(1, NTC):
        # kt tile 4*tcI first read at the band with that k-tile index
        dl = tcI * (TCW // 128)
        items = [(0.9, q) for q in kq_quanta(projp, None, tcI, scopy)]
        items.append((0.3, (lambda tcI=tcI: rope(projp, kt, tcI * TCW, TCW))))
        groups.append((dl, items))
    for ti in range(4, NT):
        # V tile ti: read by PV group qt=ti, woven at band ti+LAG
        groups.append((min(ti + LAG, NT - 1),
                       [(0.95, v_filler(projp, ti, scopy))]))
    for m in (1, 2, 3):
        for tcI in range(NTC):
            items = [(0.9, q) for q in kq_quanta(projp, m, tcI, scopy)]
            items.append(
                (0.3, (lambda m=m, tcI=tcI: rope(projp, get_qt(m),
                                                 tcI * TCW, TCW))))
            groups.append((block_base[m], items))

    sched = {}
    sched_cost = {}
    groups.sort(key=lambda x: x[0])
    g, used = 0, 0.0
    for dl, items in groups:
        for cost, fn in items:
            budget = max(0.4, cap(g) - committed(g))
            if used >= budget:
                g, used = g + 1, 0.0
                assert g < NBANDS, "filler overflow"
            assert g < dl, f"filler deadline violated: band {g} >= {dl}"
            sched.setdefault(g, []).append(fn)
            sched_cost[g] = sched_cost.get(g, 0.0) + cost
            used += cost

    # ---- attention blocks: bands with S lookahead, woven PV, per-group
    # ACT/DVE copy, keep-warm dummies ----
    pvstate = {"bank": None}
    stgs = {}

    def get_stg(m, half):
        key = (m, half)
        if key not in stgs:
            stgs[key] = stgp.tile([128, NT // 2, 130], F32, tag=f"stg{half}",
                                  name=f"stg{half}_{m}")
        return stgs[key]

    def block(bi):
        m, qlo, qhi = BLOCKS[bi]
        gbase = block_base[bi]
        top = qhi // 128 - 1
        qt0 = qlo // 128
        qtm = get_qt(m)
        ptsAs, ptsBs = {}, {}

        def emit_group(qt):
            # the two head-regions of one qt sit adjacently (130 wide) in
            # the PSUM bank ring so a single copy drains both
            if qt % 2 == 0 or pvstate["bank"] is None:
                pvstate["bank"] = pvp.tile([128, 512], F32, tag="pv",
                                           name="pv")
            bank = pvstate["bank"]
            base = 130 * (qt % 2)
            for h, pts_d in ((0, ptsAs), (1, ptsBs)):
                col = base + 65 * h
                for ki in range(qt + 1):
                    q0 = max(128 * ki, qlo)
                    nc.tensor.matmul(
                        bank[:, col:col + 65],
                        pts_d[ki][:, 128 * qt - q0:128 * qt - q0 + 128],
                        vts[ki][:, h * 65:(h + 1) * 65],
                        start=(ki == 0), stop=(ki == qt))
            half = 0 if qt < NT // 2 else 1
            stg = get_stg(m, half)
            j = qt - half * (NT // 2)
            cp = nc.scalar.copy if qt % 2 == 0 else nc.vector.tensor_copy
            cp(stg[:, j, :], bank[:, base:base + 130])
            if qt == NT // 2 - 1:
                nc.sync.dma_start(
                    out=out[:, 0:NT // 2, m * 130:(m + 1) * 130],
                    in_=get_stg(m, 0)[:])
            elif qt == (3 * NT) // 4 - 1:
                nc.sync.dma_start(
                    out=out[:, NT // 2:(3 * NT) // 4,
                            m * 130:(m + 1) * 130],
                    in_=get_stg(m, 1)[:, 0:(3 * NT) // 4 - NT // 2, :])
            elif qt == NT - 1:
                nc.sync.dma_start(
                    out=out[:, (3 * NT) // 4:NT, m * 130:(m + 1) * 130],
                    in_=get_stg(m, 1)[:, (3 * NT) // 4 - NT // 2:, :])

        def emit_S(ki):
            q0 = max(128 * ki, qlo)
            w = qhi - q0
            ptsA = ppts.tile([128, w], BF16, tag=f"pa{ki}", name=f"pa{ki}")
            ptsB = ppts.tile([128, w], BF16, tag=f"pb{ki}", name=f"pb{ki}")
            for off in range(0, w, SCW):
                cn = min(SCW, w - off)
                sA = sp.tile([128, SCW], F32, tag="s", name="sA")
                sB = sp.tile([128, SCW], F32, tag="s", name="sB")
                nc.tensor.matmul(
                    sA[:, 0:cn],
                    kt[0:64, ki * 128:(ki + 1) * 128],
                    qtm[0:64, q0 + off:q0 + off + cn],
                    start=True, stop=True)
                nc.tensor.matmul(
                    sB[:, 0:cn],
                    kt[64:128, ki * 128:(ki + 1) * 128],
                    qtm[64:128, q0 + off:q0 + off + cn],
                    start=True, stop=True)
                nc.scalar.activation(ptsA[:, off:off + cn], sA[:, 0:cn],
                                     AF.Exp, scale=SCALE)
                nc.vector.tensor_scalar(
                    out=ptsB[:, off:off + cn].bitcast(I16), in0=sB[:, 0:cn],
                    scalar1=EXP_A, scalar2=EXP_B, op0=ALU.mult, op1=ALU.add)
            if 128 * ki >= qlo:  # band contains the diagonal block
                nc.gpsimd.tensor_tensor(out=ptsA[:, 0:128],
                                        in0=ptsA[:, 0:128], in1=triu[:],
                                        op=ALU.mult)
                nc.gpsimd.tensor_tensor(out=ptsB[:, 0:128],
                                        in0=ptsB[:, 0:128], in1=triu[:],
                                        op=ALU.mult)
            ptsAs[ki], ptsBs[ki] = ptsA, ptsB

        next_s = 0
        for ki in range(top + 1):
            # S lookahead: one extra band once bands are narrow, so the
            # exp engines always have queued work while PE runs PV
            look = ki + 1 if ki + 1 <= top and \
                (qhi - 128 * (ki + 1)) <= 2 * SCW and ki >= 2 else ki
            while next_s <= min(look, top):
                emit_S(next_s)
                next_s += 1

            for f in sched.get(gbase + ki, ()):
                f()

            if ki - LAG >= qt0:
                emit_group(ki - LAG)

            # keep-warm: pad underloaded bands with dep-free matmuls into
            # the unused tail of the current PV bank
            if pvstate["bank"] is not None:
                load = committed(gbase + ki) + sched_cost.get(gbase + ki,
                                                              0.0)
                ndum = min(6, max(0, int((1.5 - load) / 0.107)))
                for _ in range(ndum):
                    nc.tensor.matmul(pvstate["bank"][:, 260:512],
                                     triu[:], kt[:, 0:252],
                                     start=True, stop=True)

        for qt in range(max(qt0, top + 1 - LAG), top + 1):
            emit_group(qt)

    for bi in range(len(BLOCKS)):
        block(bi)

    ctx.close()


def build_program(T=2048, num_devices=8):
    nc = bacc.Bacc("TRN2", target_bir_lowering=False, debug=False,
                   num_devices=num_devices)
    nd = D // 128
    xT = nc.dram_tensor("xT", (128, nd, T), BF16, kind="ExternalInput").ap()
    wqT = nc.dram_tensor("wqT", (128, nd, 512), BF16, kind="ExternalInput").ap()
    wkT = nc.dram_tensor("wkT", (128, nd, 128), BF16, kind="ExternalInput").ap()
    wvT = nc.dram_tensor("wvT", (128, nd, 128), BF16, kind="ExternalInput").ap()
    cosr = nc.dram_tensor("cosr", (128, T), BF16, kind="ExternalInput").ap()
    sins = nc.dram_tensor("sins", (128, T), BF16, kind="ExternalInput").ap()
    permM = nc.dram_tensor("perm", (128, 128), BF16, kind="ExternalInput").ap()
    # out[p, qt, c]: row qt*128+p of the logical [T, 520] output; columns
    # pair-major: pair m, head h (0=m, 1=m+4) numerator at
    # c in [130m+65h, +64), denominator at 130m+65h+64
    out = nc.dram_tensor("out", (128, T // 128, 520), F32,
                         kind="ExternalOutput").ap()
    with tile.TileContext(nc) as tc:
        _emit_body(tc, (xT, wqT, wkT, wvT, cosr, sins, permM, out), T)
    nc.compile()
    return nc


# ---------------- host side ----------------

def _qperm(j):
    rows = []
    for m in range(4):
        for r in range(128):
            h = m if r < 64 else m + 4
            d = 2 * (r % 32) + (1 if (r % 64) >= 32 else 0)
            rows.append((8 * j + h) * 64 + d)
    return np.array(rows)


def _kperm(j):
    rows = []
    for kv in range(2):
        for r in range(64):
            d = 2 * (r % 32) + (1 if r >= 32 else 0)
            rows.append((2 * j + kv) * 64 + d)
    return np.array(rows)


def _to3d(a):
    """[D, C] -> [128, D//128, C] (partition-major di stacking)."""
    Dd, C = a.shape
    return np.ascontiguousarray(a.reshape(Dd // 128, 128, C).transpose(1, 0, 2))


def _perm_mat():
    p = np.zeros((128, 128), dtype=ml_dtypes.bfloat16)
    for i in range(128):
        j = i + 32 if (i % 64) < 32 else i - 32
        p[i, j] = 1.0
    return p


def make_core_inputs(x, Wq, Wk, Wv, cos, sin):
    """Per-core input dicts (host prep). x: [B,T,D]."""
    bf = ml_dtypes.bfloat16
    B, T, _ = x.shape
    xTb = [_to3d(np.ascontiguousarray(x[b].T).astype(bf)) for b in range(B)]
    cosT = np.ascontiguousarray(cos.T.astype(np.float32))  # [32, T]
    sinT = np.ascontiguousarray(sin.T.astype(np.float32))
    cosr = np.tile(cosT, (4, 1)).astype(bf)
    sgn = np.repeat(np.array([-1.0, 1.0, -1.0, 1.0], np.float32), 32)
    sins = (np.tile(sinT, (4, 1)) * sgn[:, None]).astype(bf)
    perm = _perm_mat()
    maps = []
    for c in range(8):
        b, j = c // 4, c % 4
        maps.append({
            "xT": xTb[b],
            "wqT": _to3d(Wq[_qperm(j)].T.astype(bf)),
            "wkT": _to3d(Wk[_kperm(j)].T.astype(bf)),
            "wvT": _to3d(Wv[128 * j:128 * (j + 1)].T.astype(bf)),
            "cosr": cosr,
            "sins": sins,
            "perm": perm,
        })
    return maps


def core_out_to_full(res_out):
    """res_out: [128, NT, 520] pair-major num/den -> [T, 512] head-major."""
    nt = res_out.shape[1]
    o = np.transpose(res_out, (1, 0, 2)).reshape(nt * 128, 520)
    full = np.empty((nt * 128, 512), np.float32)
    for m in range(4):
        for h in (0, 1):
            base = m * 130 + h * 65
            num = o[:, base:base + 64]
            den = o[:, base + 64:base + 65]
            full[:, (m + 4 * h) * 64:(m + 4 * h) * 64 + 64] = num / den
    return full


_CACHE = {}


def _get_program():
    if "nc" not in _CACHE:
        _CACHE["nc"] = build_program(T=2048, num_devices=8)
    return _CACHE["nc"]


def run_on_hw(in_maps, trace=False):
    nc = _get_program()
    return run_bass_kernel_spmd(nc, in_maps, list(range(8)), trace=trace)


def kernel(x, Wq, Wk, Wv, cos, sin):
    x = np.asarray(x, np.float32)
    Wq = np.asarray(Wq, np.float32)
    Wk = np.asarray(Wk, np.float32)
    Wv = np.asarray(Wv, np.float32)
    cos = np.asarray(cos, np.float32)
    sin = np.asarray(sin, np.float32)
    maps = make_core_inputs(x, Wq, Wk, Wv, cos, sin)
    res = run_on_hw(maps, trace=False)
    B, T = x.shape[0], x.shape[1]
    out = np.empty((B, T, 2048), np.float32)
    for c in range(8):
        b, j = c // 4, c % 4
        out[b, :, 512 * j:512 * (j + 1)] = core_out_to_full(res.results[c]["out"])
    return out
